# revision 18
# baseline (speedup 1.0000x reference)
"""Distributed Trainium2 kernel for AsymmetricCausalSelfAttention (no mask).

Math (per reference):
  qkv = x @ W_attn + b_attn ; per-head scores = (q k^T) * head_temp[h]
  att = softmax(scores) ; y = (att @ v) * head_scale[h] ; out = y @ W_proj + b_proj

Sharding: head-parallel, 2 heads per core, 8 cores, no collectives.
Each core computes its 2 heads end-to-end and a partial output projection
(out_partial = y_heads @ W_proj[rows of those heads]); the host sums the 8
partials and adds b_proj.  head_temp is folded into W_q / b_q, head_scale
into W_proj rows (exact rewrites).

On-chip layout is feature-major ("transposed") throughout so every matmul
runs with moving dim 512 (full float32r rate):
  xT[C, BT] -> QT/KT/VT[128, BT] (128 = 2 heads x 64 dims)
  S.T tiles [ktok 128, qtok 512] = KT_h.T-free matmuls (contraction d=64,
     two heads packed in PE row groups 0-63 / 64-127)
  P = exp(S.T) via ScalarE (PSUM -> SBUF), tiles [128, 1024] = [h0 512|h1 512]
  O.T[64, qtok] += V[kt].T @ P tiles (two heads col-packed at array cols
     0-63 / 64-127); denominators via ones-matrix matmul -> replicated rows
  y.T = O.T * reciprocal(denom) ; out.T partial [C, BT] = Wp.T @ y.T (bf16 out)
"""

import os
import sys

sys.path.insert(0, "/opt/trn_rl_repo")

import numpy as np

B, T, C, H = 2, 2048, 1024, 16
D = C // H  # 64
NCORES = 8
HPC = H // NCORES  # 2 heads per core
NTOK = B * T  # 4096
NT_B = T  # tokens per batch
KT_PER_B = T // 128  # 16 k-token tiles per batch
QB_PER_B = T // 512  # 4 q-blocks per batch
CT = C // 128  # 8 contraction tiles for qkv
TB = NTOK // 512  # 8 token blocks for qkv/proj
OF = C // 128  # 8 output-feature tiles for proj

LAST_EXEC_NS = None
LAST_RESULTS = None

_COMPILED_NC = None


def _build():
    import concourse.bass as bass
    import concourse.tile as tile
    from concourse import mybir
    from concourse.masks import make_identity

    F32 = mybir.dt.float32
    F32R = mybir.dt.float32r
    BF16 = mybir.dt.bfloat16
    EXP = mybir.ActivationFunctionType.Exp
    IDENT = mybir.ActivationFunctionType.Identity

    nc = bass.Bass()
    xt_d = nc.declare_dram_parameter("xt", [C, NTOK], F32R, isOutput=False)
    wqkv_d = nc.declare_dram_parameter("wqkv", [128, CT, 384], F32R, isOutput=False)
    bqkv_d = nc.declare_dram_parameter("bqkv", [128, 3], F32, isOutput=False)
    wp_d = nc.declare_dram_parameter("wp", [128, C], F32R, isOutput=False)
    cones_d = nc.declare_dram_parameter("cones", [128, 80], F32R, isOutput=False)
    out_d = nc.declare_dram_parameter("out", [C, NTOK], BF16, isOutput=True)

    with tile.TileContext(nc) as tc:
        with (
            tc.tile_pool(name="consts", bufs=1) as consts,
            tc.tile_pool(name="big", bufs=1) as big,
            tc.tile_pool(name="xcol", bufs=3) as xcolp,
            tc.tile_pool(name="ptp", bufs=3) as ptp,
            tc.tile_pool(name="vecp", bufs=2) as vecp,
            tc.tile_pool(name="osbp", bufs=3) as osbp,
            tc.tile_pool(name="ps512", bufs=4, space="PSUM") as ps512,
            tc.tile_pool(name="psS", bufs=2, space="PSUM") as psS,
        ):
            # ---- constants ----
            wqkv_sb = consts.tile([128, CT, 384], F32R)
            for ct in range(CT):
                nc.gpsimd.dma_start(out=wqkv_sb[:, ct, :], in_=wqkv_d[:, ct, :])
            bqkv_sb = consts.tile([128, 3], F32)
            nc.gpsimd.dma_start(out=bqkv_sb, in_=bqkv_d[:, :])
            wp_sb = consts.tile([128, C], F32R)
            nc.gpsimd.dma_start(out=wp_sb, in_=wp_d[:, :])
            ident = consts.tile([128, 128], F32)
            make_identity(nc, ident)
            cones = consts.tile([128, 80], F32R)
            nc.gpsimd.dma_start(out=cones, in_=cones_d[:, :])

            # ---- persistent activations ----
            qt = big.tile([128, NTOK], F32R)
            ktm = big.tile([128, NTOK], F32R)
            vtm = big.tile([128, NTOK], F32)
            v_sb = big.tile([128, NTOK // 128, 130], F32R)

            # ---- phase 1: qkv projection (feature-major) ----
            xt_r = xt_d.rearrange("(ct p) t -> p ct t", p=128)  # [128, CT, NTOK]
            for tb in range(TB):
                xcol = xcolp.tile([128, CT, 512], F32R)
                for ct in range(CT):
                    nc.sync.dma_start(
                        out=xcol[:, ct, :],
                        in_=xt_r[:, ct, tb * 512 : (tb + 1) * 512],
                    )
                for wi, dest in ((0, qt), (1, ktm), (2, vtm)):
                    ps_qkv = ps512.tile([128, 512], F32, tag="acc")
                    for ct in range(CT):
                        nc.tensor.matmul(
                            ps_qkv,
                            wqkv_sb[:, ct, wi * 128 : (wi + 1) * 128],
                            xcol[:, ct, :],
                            start=(ct == 0),
                            stop=(ct == CT - 1),
                        )
                    nc.scalar.activation(
                        dest[:, tb * 512 : (tb + 1) * 512],
                        ps_qkv,
                        IDENT,
                        bias=bqkv_sb[:, wi : wi + 1],
                    )

            # ---- phase 1.5: transpose V to token-major; append ones columns ----
            nc.vector.tensor_copy(v_sb[:, :, 64], cones[:, 0:32])
            nc.vector.tensor_copy(v_sb[:, :, 129], cones[:, 32:64])
            for ktg in range(NTOK // 128):
                ps_tr = ps512.tile([128, 512], F32, tag="acc")
                nc.tensor.transpose(
                    ps_tr[:, 0:128], vtm[:, ktg * 128 : (ktg + 1) * 128], ident
                )
                nc.vector.tensor_copy(v_sb[:, ktg, 0:64], ps_tr[:, 0:64])
                nc.vector.tensor_copy(v_sb[:, ktg, 65:129], ps_tr[:, 64:128])

            # ---- phase 2+3: attention + partial projection ----
            # The projection of q-block N is emitted after the attention
            # kt-loop of q-block N+1 so the PE never stalls on the softmax
            # normalization chain (copy -> bcast-matmul -> reciprocal -> mul).
            def emit_proj(yt_prev, qsl_prev):
                for of in range(OF):
                    ps_pr = ps512.tile([128, 512], F32, tag="acc")
                    nc.tensor.matmul(
                        ps_pr,
                        wp_sb[:, of * 128 : (of + 1) * 128],
                        yt_prev,
                        start=True,
                        stop=True,
                    )
                    ob = osbp.tile([128, 512], BF16, tag="ob")
                    nc.vector.tensor_copy(ob, ps_pr)
                    nc.sync.dma_start(
                        out=out_d[of * 128 : (of + 1) * 128, qsl_prev], in_=ob
                    )

            pending_proj = None
            for b in range(B):
                for qb in range(QB_PER_B):
                    col0 = b * NT_B + qb * 512
                    qsl = slice(col0, col0 + 512)
                    # rows 0:64 = attention numerator, row 64 = softmax denom
                    ot0 = ps512.tile([128, 512], F32, tag="acc")
                    ot1 = ps512.tile([128, 512], F32, tag="acc")
                    for kt in range(KT_PER_B):
                        krow0 = b * NT_B + kt * 128
                        ksl = slice(krow0, krow0 + 128)
                        s_ps = psS.tile([128, 1024], F32, tag="s")
                        # scores (transposed): two heads row-packed in the PE
                        nc.tensor.matmul(
                            s_ps[:, 0:512],
                            ktm[0:64, ksl],
                            qt[0:64, qsl],
                            start=True,
                            stop=True,
                            tile_position=(0, 0),
                        )
                        nc.tensor.matmul(
                            s_ps[:, 512:1024],
                            ktm[64:128, ksl],
                            qt[64:128, qsl],
                            start=True,
                            stop=True,
                            tile_position=(64, 0),
                        )
                        pt = ptp.tile([128, 1024], F32R, tag="pt")
                        nc.scalar.activation(pt, s_ps, EXP)
                        vk = b * KT_PER_B + kt
                        first = kt == 0
                        last = kt == KT_PER_B - 1
                        nc.tensor.matmul(
                            ot0[0:65, :],
                            v_sb[:, vk, 0:65],
                            pt[:, 0:512],
                            start=first,
                            stop=last,
                        )
                        nc.tensor.matmul(
                            ot1[0:65, :],
                            v_sb[:, vk, 65:130],
                            pt[:, 512:1024],
                            start=first,
                            stop=last,
                        )
                    if pending_proj is not None:
                        emit_proj(*pending_proj)
                    # normalize: copy OT+denom to SBUF immediately (frees the
                    # PSUM accumulators), K=1 ones-matmul broadcast of the
                    # denom row -> reciprocal -> multiply; h1 partition-shift
                    yt = vecp.tile([128, 512], F32R, tag="yt")
                    ytmp = vecp.tile([128, 512], F32R, tag="ytmp")
                    ot0_sb = vecp.tile([128, 512], F32R, tag="ot0_sb")
                    ot1_sb = vecp.tile([128, 512], F32R, tag="ot1_sb")
                    nc.vector.tensor_copy(ot0_sb[0:65, :], ot0[0:65, :])
                    nc.vector.tensor_copy(ot1_sb[0:65, :], ot1[0:65, :])
                    for h, ot_sb in ((0, ot0_sb), (1, ot1_sb)):
                        dn = ps512.tile([128, 512], F32, tag="acc")
                        nc.tensor.matmul(
                            dn[0:65, :],
                            cones[64:65, 0:65],
                            ot_sb[64:65, :],
                            start=True,
                            stop=True,
                        )
                        rd = vecp.tile([128, 512], F32, tag="rd")
                        nc.vector.reciprocal(rd[0:64, :], dn[0:64, :])
                        if h == 0:
                            nc.vector.tensor_mul(yt[0:64, :], ot_sb[0:64, :], rd[0:64, :])
                        else:
                            nc.vector.tensor_mul(
                                ytmp[0:64, :], ot_sb[0:64, :], rd[0:64, :]
                            )
                    nc.sync.dma_start(out=yt[64:128, :], in_=ytmp[0:64, :])
                    pending_proj = (yt, qsl)
            emit_proj(*pending_proj)

    # Several TRN2 instruction structs (self-loading fp32r matmult LDWEIGHTS,
    # TensorScalarPtr, ...) can carry only one sync wait; Tile sometimes
    # schedules 2+. Peel excess waits onto no-ops inserted just before the
    # instruction on the same engine (same engine => same FIFO order).
    compute_engines = {
        mybir.EngineType.PE,
        mybir.EngineType.DVE,
        mybir.EngineType.Activation,
        mybir.EngineType.Pool,
        mybir.EngineType.SP,
    }
    for blk in nc.m.functions[0].blocks:
        insts = blk.instructions
        i = 0
        while i < len(insts):
            inst = insts[i]
            if (
                inst.opcode not in ("NoOp", "AllEngineBarrier")
                and inst.engine in compute_engines
                and inst.sync_info is not None
            ):
                waits = list(inst.sync_info.on_wait)
                if len(waits) > 1:
                    for j, w in enumerate(waits[:-1]):
                        nop = mybir.InstNoOp(
                            name=f"{inst.name}_waitnop{j}",
                            engine=inst.engine,
                            ins=[],
                            outs=[],
                        )
                        nop.sync_info = mybir.SyncInfo(on_wait=[w], on_update=[])
                        insts.insert(i, nop)
                        i += 1
                    inst.sync_info = mybir.SyncInfo(
                        on_wait=[waits[-1]], on_update=list(inst.sync_info.on_update)
                    )
            i += 1
    return nc


_LDW_PATCHED = False


def _maybe_patch_ldw_opt():
    """Optionally flip walrus --enable-ldw-opt (env KERNEL_LDW_OPT=1)."""
    global _LDW_PATCHED
    if _LDW_PATCHED or not os.environ.get("KERNEL_LDW_OPT"):
        return
    from concourse import bass_utils

    orig = bass_utils.run_command

    def patched(cmd, *a, **kw):
        if isinstance(cmd, list):
            cmd = [
                c.replace("--enable-ldw-opt=false", "--enable-ldw-opt=true")
                if isinstance(c, str)
                else c
                for c in cmd
            ]
        return orig(cmd, *a, **kw)

    bass_utils.run_command = patched
    _LDW_PATCHED = True


def _install_trace_shim():
    """Provide antenv.axon_hooks + a no-op artifact upload so that
    run_bass_kernel_spmd(trace=True) can capture NTFF profiles under axon.
    Returns True if the hook could be installed."""
    try:
        import types

        import antenv
        from concourse import bass_utils

        bass_utils.upload_artifacts = lambda tmpdir: str(tmpdir)
        if "antenv.axon_hooks" not in sys.modules:
            mod = types.ModuleType("antenv.axon_hooks")
            state = {"hook": None}

            def set_axon_ntff_profile_hook(h):
                state["hook"] = h

            def get_axon_ntff_profile_hook():
                return state["hook"]

            mod.set_axon_ntff_profile_hook = set_axon_ntff_profile_hook
            mod.get_axon_ntff_profile_hook = get_axon_ntff_profile_hook
            sys.modules["antenv.axon_hooks"] = mod
            antenv.axon_hooks = mod
        from antenv.axon_hooks import (
            get_axon_ntff_profile_hook,
            set_axon_ntff_profile_hook,
        )

        if get_axon_ntff_profile_hook() is None:
            from trn_agent_boot.trn_boot import _ntff_profile_via_ctypes

            hook = _ntff_profile_via_ctypes("/opt/axon/libaxon_pjrt.so")
            if hook is None:
                return False
            set_axon_ntff_profile_hook(hook)
        return True
    except Exception as e:  # pragma: no cover - tracing is best-effort
        print(f"trace shim install failed: {e}", file=sys.stderr)
        return False


def _get_compiled():
    global _COMPILED_NC
    if _COMPILED_NC is None:
        _COMPILED_NC = _build()
    return _COMPILED_NC


def kernel(x, W_attn, b_attn, head_temp, head_scale, W_proj, b_proj):
    global LAST_EXEC_NS, LAST_RESULTS
    x = np.asarray(x, dtype=np.float32)
    W_attn = np.asarray(W_attn, dtype=np.float32)
    b_attn = np.asarray(b_attn, dtype=np.float32)
    head_temp = np.asarray(head_temp, dtype=np.float32)
    head_scale = np.asarray(head_scale, dtype=np.float32)
    W_proj = np.asarray(W_proj, dtype=np.float32)
    b_proj = np.asarray(b_proj, dtype=np.float32)

    nc = _get_compiled()

    xt = np.ascontiguousarray(x.reshape(NTOK, C).T)  # [C, NTOK]
    in_maps = []
    for c in range(NCORES):
        cs = slice(128 * c, 128 * (c + 1))
        tempvec = np.repeat(head_temp[HPC * c : HPC * (c + 1)], D)  # [128]
        scalevec = np.repeat(head_scale[HPC * c : HPC * (c + 1)], D)  # [128]
        wq = W_attn[:, cs] * tempvec[None, :]
        wk = W_attn[:, C:][:, cs]
        wv = W_attn[:, 2 * C :][:, cs]
        wqkv = np.concatenate([wq, wk, wv], axis=1)  # [1024, 384]
        wqkv = np.ascontiguousarray(
            wqkv.reshape(CT, 128, 384).transpose(1, 0, 2)
        )  # [128, CT, 384]
        bq = b_attn[cs] * tempvec
        bk = b_attn[C:][cs]
        bv = b_attn[2 * C :][cs]
        bqkv = np.ascontiguousarray(np.stack([bq, bk, bv], axis=1))  # [128, 3]
        wp = np.ascontiguousarray(W_proj[cs, :] * scalevec[:, None])  # [128, C]
        cones = np.ones((128, 80), dtype=np.float32)
        in_maps.append(
            {"xt": xt, "wqkv": wqkv, "bqkv": bqkv, "wp": wp, "cones": cones}
        )

    from concourse.bass_utils import run_bass_kernel_spmd

    _maybe_patch_ldw_opt()
    trace = bool(os.environ.get("KERNEL_TRACE"))
    tmpdir = os.environ.get("KERNEL_TRACE_DIR") or None
    if trace:
        trace = _install_trace_shim()
    res = run_bass_kernel_spmd(
        nc, in_maps, list(range(NCORES)), trace=trace, tmpdir=tmpdir
    )
    LAST_EXEC_NS = res.exec_time_ns
    LAST_RESULTS = res

    acc = np.zeros((C, NTOK), dtype=np.float32)
    for i in range(NCORES):
        acc += np.asarray(res.results[i]["out"]).astype(np.float32)
    out = acc.T.reshape(B, T, C) + b_proj[None, None, :]
    return out.astype(np.float32)


# revision 19
# speedup vs baseline: 1.1803x; 1.1803x over previous
"""Distributed Trainium2 kernel for AsymmetricCausalSelfAttention (no mask).

Math (per reference):
  qkv = x @ W_attn + b_attn ; per-head scores = (q k^T) * head_temp[h]
  att = softmax(scores) ; y = (att @ v) * head_scale[h] ; out = y @ W_proj + b_proj

Sharding: head-parallel, 2 heads per core, 8 cores, no collectives.
Each core computes its 2 heads end-to-end and a partial output projection
(out_partial = y_heads @ W_proj[rows of those heads]); the host sums the 8
partials and adds b_proj.  head_temp is folded into W_q / b_q, head_scale
into W_proj rows (exact rewrites).

On-chip layout is feature-major ("transposed") throughout so every matmul
runs with moving dim 512 (full float32r rate):
  xT[C, BT] -> QT/KT/VT[128, BT] (128 = 2 heads x 64 dims)
  S.T tiles [ktok 128, qtok 512] = KT_h.T-free matmuls (contraction d=64,
     two heads packed in PE row groups 0-63 / 64-127)
  P = exp(S.T) via ScalarE (PSUM -> SBUF), tiles [128, 1024] = [h0 512|h1 512]
  O.T[64, qtok] += V[kt].T @ P tiles (two heads col-packed at array cols
     0-63 / 64-127); denominators via ones-matrix matmul -> replicated rows
  y.T = O.T * reciprocal(denom) ; out.T partial [C, BT] = Wp.T @ y.T (bf16 out)
"""

import os
import sys

sys.path.insert(0, "/opt/trn_rl_repo")

import numpy as np

B, T, C, H = 2, 2048, 1024, 16
D = C // H  # 64
NCORES = 8
HPC = H // NCORES  # 2 heads per core
NTOK = B * T  # 4096
NT_B = T  # tokens per batch
KT_PER_B = T // 128  # 16 k-token tiles per batch
QB_PER_B = T // 512  # 4 q-blocks per batch
CT = C // 128  # 8 contraction tiles for qkv
TB = NTOK // 512  # 8 token blocks for qkv/proj
OF = C // 128  # 8 output-feature tiles for proj

LAST_EXEC_NS = None
LAST_RESULTS = None

_COMPILED_NC = None


def _build():
    import concourse.bass as bass
    import concourse.tile as tile
    from concourse import mybir
    from concourse.masks import make_identity

    F32 = mybir.dt.float32
    F32R = mybir.dt.float32r
    BF16 = mybir.dt.bfloat16
    EXP = mybir.ActivationFunctionType.Exp
    IDENT = mybir.ActivationFunctionType.Identity

    nc = bass.Bass()
    xt_d = nc.declare_dram_parameter("xt", [C, NTOK], F32R, isOutput=False)
    wqkv_d = nc.declare_dram_parameter("wqkv", [128, CT, 384], F32R, isOutput=False)
    bqkv_d = nc.declare_dram_parameter("bqkv", [128, 3], F32, isOutput=False)
    wp_d = nc.declare_dram_parameter("wp", [128, C], F32R, isOutput=False)
    cones_d = nc.declare_dram_parameter("cones", [128, 80], F32R, isOutput=False)
    out_d = nc.declare_dram_parameter("out", [C, NTOK], BF16, isOutput=True)

    with tile.TileContext(nc) as tc:
        with (
            tc.tile_pool(name="consts", bufs=1) as consts,
            tc.tile_pool(name="big", bufs=1) as big,
            tc.tile_pool(name="xcol", bufs=3) as xcolp,
            tc.tile_pool(name="ptp", bufs=3) as ptp,
            tc.tile_pool(name="vecp", bufs=3) as vecp,
            tc.tile_pool(name="osbp", bufs=3) as osbp,
            tc.tile_pool(name="ps512", bufs=4, space="PSUM") as ps512,
            tc.tile_pool(name="psS", bufs=2, space="PSUM") as psS,
        ):
            # ---- constants ----
            wqkv_sb = consts.tile([128, CT, 384], F32R)
            for ct in range(CT):
                nc.gpsimd.dma_start(out=wqkv_sb[:, ct, :], in_=wqkv_d[:, ct, :])
            bqkv_sb = consts.tile([128, 3], F32)
            nc.gpsimd.dma_start(out=bqkv_sb, in_=bqkv_d[:, :])
            wp_sb = consts.tile([128, C], F32R)
            nc.gpsimd.dma_start(out=wp_sb, in_=wp_d[:, :])
            ident = consts.tile([128, 128], F32)
            make_identity(nc, ident)
            cones = consts.tile([128, 80], F32R)
            nc.gpsimd.dma_start(out=cones, in_=cones_d[:, :])

            # ---- persistent activations ----
            qt = big.tile([128, NTOK], F32R)
            ktm = big.tile([128, NTOK], F32R)
            vtm = big.tile([128, NTOK], F32)
            v_sb = big.tile([128, NTOK // 128, 130], F32R)

            # ---- phase 1: qkv projection (feature-major) ----
            xt_r = xt_d.rearrange("(ct p) t -> p ct t", p=128)  # [128, CT, NTOK]
            for tb in range(TB):
                xcol = xcolp.tile([128, CT, 512], F32R)
                for ct in range(CT):
                    nc.sync.dma_start(
                        out=xcol[:, ct, :],
                        in_=xt_r[:, ct, tb * 512 : (tb + 1) * 512],
                    )
                for wi, dest in ((0, qt), (1, ktm), (2, vtm)):
                    ps_qkv = ps512.tile([128, 512], F32, tag="acc")
                    for ct in range(CT):
                        nc.tensor.matmul(
                            ps_qkv,
                            wqkv_sb[:, ct, wi * 128 : (wi + 1) * 128],
                            xcol[:, ct, :],
                            start=(ct == 0),
                            stop=(ct == CT - 1),
                        )
                    nc.scalar.activation(
                        dest[:, tb * 512 : (tb + 1) * 512],
                        ps_qkv,
                        IDENT,
                        bias=bqkv_sb[:, wi : wi + 1],
                    )

            # ---- phase 1.5: transpose V to token-major; append ones columns ----
            nc.vector.tensor_copy(v_sb[:, :, 64], cones[:, 0:32])
            nc.vector.tensor_copy(v_sb[:, :, 129], cones[:, 32:64])
            for ktg in range(NTOK // 128):
                ps_tr = ps512.tile([128, 512], F32, tag="acc")
                nc.tensor.transpose(
                    ps_tr[:, 0:128], vtm[:, ktg * 128 : (ktg + 1) * 128], ident
                )
                nc.vector.tensor_copy(v_sb[:, ktg, 0:64], ps_tr[:, 0:64])
                nc.vector.tensor_copy(v_sb[:, ktg, 65:129], ps_tr[:, 64:128])

            # ---- phase 2+3: attention + partial projection ----
            # Two-deep software pipeline: the normalization of q-block N-1 and
            # the projection of q-block N-2 are emitted one task per kt-step
            # inside q-block N's attention loop, so every PE instruction's
            # dependencies are satisfied ~a full iteration in advance and the
            # PE never sees head-of-line blocking (which would let the HAM
            # clock gate re-throttle it).
            def make_norm_tasks(st):
                tasks = []
                for h in (0, 1):
                    def t(h=h, st=st):
                        ot_sb = st["ot0_sb"] if h == 0 else st["ot1_sb"]
                        dn = ps512.tile([128, 512], F32, tag="acc")
                        nc.tensor.matmul(
                            dn[0:65, :],
                            cones[64:65, 0:65],
                            ot_sb[64:65, :],
                            start=True,
                            stop=True,
                        )
                        rd = vecp.tile([128, 512], F32, tag="rd")
                        nc.vector.reciprocal(rd[0:64, :], dn[0:64, :])
                        dst = st["yt"] if h == 0 else st["ytmp"]
                        nc.vector.tensor_mul(
                            dst[0:64, :], ot_sb[0:64, :], rd[0:64, :]
                        )
                    tasks.append(t)

                def tshift(st=st):
                    nc.sync.dma_start(
                        out=st["yt"][64:128, :], in_=st["ytmp"][0:64, :]
                    )
                tasks.append(tshift)
                return tasks

            def make_proj_tasks(st):
                tasks = []
                for of in range(OF):
                    def t(of=of, st=st):
                        ps_pr = ps512.tile([128, 512], F32, tag="acc")
                        nc.tensor.matmul(
                            ps_pr,
                            wp_sb[:, of * 128 : (of + 1) * 128],
                            st["yt"],
                            start=True,
                            stop=True,
                        )
                        ob = osbp.tile([128, 512], BF16, tag="ob")
                        nc.vector.tensor_copy(ob, ps_pr)
                        nc.sync.dma_start(
                            out=out_d[of * 128 : (of + 1) * 128, st["qsl"]], in_=ob
                        )
                    tasks.append(t)
                return tasks

            pending = []
            norm_prev = None  # state of q-block N-1 (awaiting normalization)
            for b in range(B):
                for qb in range(QB_PER_B):
                    col0 = b * NT_B + qb * 512
                    qsl = slice(col0, col0 + 512)
                    # rows 0:64 = attention numerator, row 64 = softmax denom
                    ot0 = ps512.tile([128, 512], F32, tag="acc")
                    ot1 = ps512.tile([128, 512], F32, tag="acc")
                    for kt in range(KT_PER_B):
                        krow0 = b * NT_B + kt * 128
                        ksl = slice(krow0, krow0 + 128)
                        s_ps = psS.tile([128, 1024], F32, tag="s")
                        # scores (transposed): two heads row-packed in the PE
                        nc.tensor.matmul(
                            s_ps[:, 0:512],
                            ktm[0:64, ksl],
                            qt[0:64, qsl],
                            start=True,
                            stop=True,
                            tile_position=(0, 0),
                        )
                        nc.tensor.matmul(
                            s_ps[:, 512:1024],
                            ktm[64:128, ksl],
                            qt[64:128, qsl],
                            start=True,
                            stop=True,
                            tile_position=(64, 0),
                        )
                        pt = ptp.tile([128, 1024], F32R, tag="pt")
                        nc.scalar.activation(pt, s_ps, EXP)
                        vk = b * KT_PER_B + kt
                        first = kt == 0
                        last = kt == KT_PER_B - 1
                        nc.tensor.matmul(
                            ot0[0:65, :],
                            v_sb[:, vk, 0:65],
                            pt[:, 0:512],
                            start=first,
                            stop=last,
                        )
                        nc.tensor.matmul(
                            ot1[0:65, :],
                            v_sb[:, vk, 65:130],
                            pt[:, 512:1024],
                            start=first,
                            stop=last,
                        )
                        if pending:
                            pending.pop(0)()
                    while pending:
                        pending.pop(0)()
                    # copy OT+denom to SBUF immediately (frees the PSUM
                    # accumulators for the next q-block)
                    yt = vecp.tile([128, 512], F32R, tag="yt")
                    ytmp = vecp.tile([128, 512], F32R, tag="ytmp")
                    ot0_sb = vecp.tile([128, 512], F32R, tag="ot0_sb")
                    ot1_sb = vecp.tile([128, 512], F32R, tag="ot1_sb")
                    nc.vector.tensor_copy(ot0_sb[0:65, :], ot0[0:65, :])
                    nc.vector.tensor_copy(ot1_sb[0:65, :], ot1[0:65, :])
                    st = {
                        "yt": yt,
                        "ytmp": ytmp,
                        "ot0_sb": ot0_sb,
                        "ot1_sb": ot1_sb,
                        "qsl": qsl,
                    }
                    pending = make_norm_tasks(st)
                    if norm_prev is not None:
                        pending += make_proj_tasks(norm_prev)
                    norm_prev = st
            for t in pending:
                t()
            for t in make_proj_tasks(norm_prev):
                t()

    # Several TRN2 instruction structs (self-loading fp32r matmult LDWEIGHTS,
    # TensorScalarPtr, ...) can carry only one sync wait; Tile sometimes
    # schedules 2+. Peel excess waits onto no-ops inserted just before the
    # instruction on the same engine (same engine => same FIFO order).
    compute_engines = {
        mybir.EngineType.PE,
        mybir.EngineType.DVE,
        mybir.EngineType.Activation,
        mybir.EngineType.Pool,
        mybir.EngineType.SP,
    }
    for blk in nc.m.functions[0].blocks:
        insts = blk.instructions
        i = 0
        while i < len(insts):
            inst = insts[i]
            if (
                inst.opcode not in ("NoOp", "AllEngineBarrier")
                and inst.engine in compute_engines
                and inst.sync_info is not None
            ):
                waits = list(inst.sync_info.on_wait)
                if len(waits) > 1:
                    for j, w in enumerate(waits[:-1]):
                        nop = mybir.InstNoOp(
                            name=f"{inst.name}_waitnop{j}",
                            engine=inst.engine,
                            ins=[],
                            outs=[],
                        )
                        nop.sync_info = mybir.SyncInfo(on_wait=[w], on_update=[])
                        insts.insert(i, nop)
                        i += 1
                    inst.sync_info = mybir.SyncInfo(
                        on_wait=[waits[-1]], on_update=list(inst.sync_info.on_update)
                    )
            i += 1
    return nc


_LDW_PATCHED = False


def _maybe_patch_ldw_opt():
    """Optionally flip walrus --enable-ldw-opt (env KERNEL_LDW_OPT=1)."""
    global _LDW_PATCHED
    if _LDW_PATCHED or not os.environ.get("KERNEL_LDW_OPT"):
        return
    from concourse import bass_utils

    orig = bass_utils.run_command

    def patched(cmd, *a, **kw):
        if isinstance(cmd, list):
            cmd = [
                c.replace("--enable-ldw-opt=false", "--enable-ldw-opt=true")
                if isinstance(c, str)
                else c
                for c in cmd
            ]
        return orig(cmd, *a, **kw)

    bass_utils.run_command = patched
    _LDW_PATCHED = True


def _install_trace_shim():
    """Provide antenv.axon_hooks + a no-op artifact upload so that
    run_bass_kernel_spmd(trace=True) can capture NTFF profiles under axon.
    Returns True if the hook could be installed."""
    try:
        import types

        import antenv
        from concourse import bass_utils

        bass_utils.upload_artifacts = lambda tmpdir: str(tmpdir)
        if "antenv.axon_hooks" not in sys.modules:
            mod = types.ModuleType("antenv.axon_hooks")
            state = {"hook": None}

            def set_axon_ntff_profile_hook(h):
                state["hook"] = h

            def get_axon_ntff_profile_hook():
                return state["hook"]

            mod.set_axon_ntff_profile_hook = set_axon_ntff_profile_hook
            mod.get_axon_ntff_profile_hook = get_axon_ntff_profile_hook
            sys.modules["antenv.axon_hooks"] = mod
            antenv.axon_hooks = mod
        from antenv.axon_hooks import (
            get_axon_ntff_profile_hook,
            set_axon_ntff_profile_hook,
        )

        if get_axon_ntff_profile_hook() is None:
            from trn_agent_boot.trn_boot import _ntff_profile_via_ctypes

            hook = _ntff_profile_via_ctypes("/opt/axon/libaxon_pjrt.so")
            if hook is None:
                return False
            set_axon_ntff_profile_hook(hook)
        return True
    except Exception as e:  # pragma: no cover - tracing is best-effort
        print(f"trace shim install failed: {e}", file=sys.stderr)
        return False


def _get_compiled():
    global _COMPILED_NC
    if _COMPILED_NC is None:
        _COMPILED_NC = _build()
    return _COMPILED_NC


def kernel(x, W_attn, b_attn, head_temp, head_scale, W_proj, b_proj):
    global LAST_EXEC_NS, LAST_RESULTS
    x = np.asarray(x, dtype=np.float32)
    W_attn = np.asarray(W_attn, dtype=np.float32)
    b_attn = np.asarray(b_attn, dtype=np.float32)
    head_temp = np.asarray(head_temp, dtype=np.float32)
    head_scale = np.asarray(head_scale, dtype=np.float32)
    W_proj = np.asarray(W_proj, dtype=np.float32)
    b_proj = np.asarray(b_proj, dtype=np.float32)

    nc = _get_compiled()

    xt = np.ascontiguousarray(x.reshape(NTOK, C).T)  # [C, NTOK]
    in_maps = []
    for c in range(NCORES):
        cs = slice(128 * c, 128 * (c + 1))
        tempvec = np.repeat(head_temp[HPC * c : HPC * (c + 1)], D)  # [128]
        scalevec = np.repeat(head_scale[HPC * c : HPC * (c + 1)], D)  # [128]
        wq = W_attn[:, cs] * tempvec[None, :]
        wk = W_attn[:, C:][:, cs]
        wv = W_attn[:, 2 * C :][:, cs]
        wqkv = np.concatenate([wq, wk, wv], axis=1)  # [1024, 384]
        wqkv = np.ascontiguousarray(
            wqkv.reshape(CT, 128, 384).transpose(1, 0, 2)
        )  # [128, CT, 384]
        bq = b_attn[cs] * tempvec
        bk = b_attn[C:][cs]
        bv = b_attn[2 * C :][cs]
        bqkv = np.ascontiguousarray(np.stack([bq, bk, bv], axis=1))  # [128, 3]
        wp = np.ascontiguousarray(W_proj[cs, :] * scalevec[:, None])  # [128, C]
        cones = np.ones((128, 80), dtype=np.float32)
        in_maps.append(
            {"xt": xt, "wqkv": wqkv, "bqkv": bqkv, "wp": wp, "cones": cones}
        )

    from concourse.bass_utils import run_bass_kernel_spmd

    _maybe_patch_ldw_opt()
    trace = bool(os.environ.get("KERNEL_TRACE"))
    tmpdir = os.environ.get("KERNEL_TRACE_DIR") or None
    if trace:
        trace = _install_trace_shim()
    res = run_bass_kernel_spmd(
        nc, in_maps, list(range(NCORES)), trace=trace, tmpdir=tmpdir
    )
    LAST_EXEC_NS = res.exec_time_ns
    LAST_RESULTS = res

    acc = np.zeros((C, NTOK), dtype=np.float32)
    for i in range(NCORES):
        acc += np.asarray(res.results[i]["out"]).astype(np.float32)
    out = acc.T.reshape(B, T, C) + b_proj[None, None, :]
    return out.astype(np.float32)
